# revision 14
# baseline (speedup 1.0000x reference)
"""DeepseekV3 MLA attention prefill (S=1024, H=128 heads, HID=7168) on 8 TRN2
NeuronCores.

Sharding: tensor-parallel over heads (16 heads/core); the low-rank input
projections (q_a / kv_a) are sequence-sharded (128 rows/core) and exchanged
with one AllGather of the rms-normed activations (natural [s, col] layout so
the collective moves 4KB rows). Each core emits a partial output projection
(contraction over its own 16 heads, transposed [HID, S] layout so each ow
stationary tile serves two matmuls); the host sums the 8 partials.

All matmul operands are bf16 (weights pre-cast on host, activations cast at
the psum->SBUF copy); softmax/rmsnorm math stays f32. The attention inner
loop is software-pipelined: AV matmuls run one kc-step behind the score
matmuls and the softmax normalization matmuls are deferred into the next
qt/group's instruction stream so exp/mask/reduce latency never stalls the
in-order PE queue. Post-AllGather stitching uses DMA-engine transposes.
"""
import math
import numpy as np
import ml_dtypes

import concourse.bass as bass
import concourse.mybir as mybir
import concourse.bacc as bacc
import concourse.tile as tile
import concourse.bass_utils as bass_utils
from contextlib import ExitStack

F32 = mybir.dt.float32
BF16 = mybir.dt.bfloat16
AF = mybir.ActivationFunctionType
OP = mybir.AluOpType

N_CORES = 8
S = 1024
HID = 7168
H = 128
HG = H // N_CORES          # 16 heads per core
D_NOPE = 128
D_ROPE = 64
D_Q = D_NOPE + D_ROPE      # 192
D_V = 128
CQ = 1536                  # q lora rank
CKV = 512                  # kv lora rank
CA = CQ + CKV + D_ROPE     # 2112 fused a-proj cols
S_SH = S // N_CORES        # 128 sequence rows per core
CC_A = HID // 128          # 56 contraction chunks for a-proj
NT_A = [(0, 512), (512, 512), (1024, 512), (1536, 512), (2048, 64)]
SCALE = 1.0 / math.sqrt(D_Q)
EPS = 1e-6
G_HEADS = 2                # heads per group
N_GROUPS = HG // G_HEADS   # 8 groups
LAST_EXEC_NS = None

_CACHE = {}


def _dma_rows_to_3d(nc, dst, src_ap, n_chunks, p=128):
    """dst [p, n_chunks, w] <- src rows laid out as (chunk, p)."""
    try:
        nc.sync.dma_start(dst, src_ap.rearrange("(c p) s -> p c s", p=p))
    except Exception:
        for c in range(n_chunks):
            nc.sync.dma_start(dst[:, c, :], src_ap[c * p:(c + 1) * p, :])


def _build_nc():
    nc = bacc.Bacc("TRN2", target_bir_lowering=False, debug=False,
                   num_devices=N_CORES)

    xT = nc.dram_tensor("xT", [HID, S_SH], BF16, kind="ExternalInput")
    wa = nc.dram_tensor("wa", [HID, CA], BF16, kind="ExternalInput")
    qbn = nc.dram_tensor("qbn", [CQ, HG * D_NOPE], BF16, kind="ExternalInput")
    qbp = nc.dram_tensor("qbp", [CQ, HG * D_ROPE], BF16, kind="ExternalInput")
    kvbk = nc.dram_tensor("kvbk", [CKV, HG * D_NOPE], BF16, kind="ExternalInput")
    kvbv = nc.dram_tensor("kvbv", [CKV, HG * D_V], BF16, kind="ExternalInput")
    ow = nc.dram_tensor("ow", [HG * D_V, HID], BF16, kind="ExternalInput")
    cos_s = nc.dram_tensor("cos_s", [S_SH, D_ROPE], F32, kind="ExternalInput")
    sin_sg = nc.dram_tensor("sin_sg", [S_SH, D_ROPE], F32, kind="ExternalInput")
    cos2t = nc.dram_tensor("cos2t", [128, S], F32, kind="ExternalInput")
    sin2tg = nc.dram_tensor("sin2tg", [128, S], F32, kind="ExternalInput")
    masks = nc.dram_tensor("masks", [512, 512], BF16, kind="ExternalInput")
    ones_col = nc.dram_tensor("ones_col", [128, 1], BF16, kind="ExternalInput")
    ones_row = nc.dram_tensor("ones_row", [1, 128], BF16, kind="ExternalInput")
    outT = nc.dram_tensor("outT", [HID, S], BF16, kind="ExternalOutput")

    with tile.TileContext(nc) as tc, ExitStack() as top:
        const = top.enter_context(tc.tile_pool(name="const", bufs=1))
        dram = top.enter_context(tc.tile_pool(name="dram", bufs=1, space="DRAM"))
        outsp = top.enter_context(tc.tile_pool(name="outsp", bufs=1))
        # phase B/C weight + staging pools opened at top level so their
        # prefetch DMAs can be emitted before the AllGather
        sbwq = top.enter_context(tc.tile_pool(name="sbwq", bufs=2))
        sbow = top.enter_context(tc.tile_pool(name="sbow", bufs=2))

        # ---- constants in SBUF ----
        masks_sb = const.tile([128, 4, 512], BF16, tag="masks")
        _dma_rows_to_3d(nc, masks_sb[:], masks.ap(), 4)
        cos_s_sb = const.tile([S_SH, D_ROPE], F32, tag="coss")
        sin_sg_sb = const.tile([S_SH, D_ROPE], F32, tag="sinsg")
        nc.sync.dma_start(cos_s_sb[:], cos_s.ap())
        nc.sync.dma_start(sin_sg_sb[:], sin_sg.ap())
        cos2t_sb = const.tile([128, S], F32, tag="cos2t")
        sin2tg_sb = const.tile([128, S], F32, tag="sin2tg")
        nc.sync.dma_start(cos2t_sb[:], cos2t.ap())
        nc.sync.dma_start(sin2tg_sb[:], sin2tg.ap())
        ones_col_sb = const.tile([128, 1], BF16, tag="onesc")
        ones_row_sb = const.tile([1, 128], BF16, tag="onesr")
        nc.sync.dma_start(ones_col_sb[:], ones_col.ap())
        nc.sync.dma_start(ones_row_sb[:], ones_row.ap())

        agi = dram.tile([S_SH, CA], BF16, tag="agi")
        ago = dram.tile([S, CA], BF16, tag="ago", addr_space="Shared")

        # all 16 heads' attention outputs live in SBUF [dv=128, head, s]
        outs_sb = outsp.tile([128, HG, S], BF16, tag="outs")

        def load_group_weights(g):
            h0 = g * G_HEADS
            qbnw = sbwq.tile([128, CQ // 128, G_HEADS * 128], BF16,
                             tag="qbnw", name="qbnw")
            qbpw = sbwq.tile([128, CQ // 128, G_HEADS * 64], BF16,
                             tag="qbpw", name="qbpw")
            kvbkw = sbwq.tile([128, CKV // 128, G_HEADS * 128], BF16,
                              tag="kvbkw", name="kvbkw")
            kvbvw = sbwq.tile([128, CKV // 128, G_HEADS * 128], BF16,
                              tag="kvbvw", name="kvbvw")
            _dma_rows_to_3d(nc, qbnw[:],
                            qbn.ap()[:, h0 * 128:(h0 + G_HEADS) * 128], CQ // 128)
            _dma_rows_to_3d(nc, qbpw[:],
                            qbp.ap()[:, h0 * 64:(h0 + G_HEADS) * 64], CQ // 128)
            _dma_rows_to_3d(nc, kvbkw[:],
                            kvbk.ap()[:, h0 * 128:(h0 + G_HEADS) * 128], CKV // 128)
            _dma_rows_to_3d(nc, kvbvw[:],
                            kvbv.ap()[:, h0 * 128:(h0 + G_HEADS) * 128], CKV // 128)
            return qbnw, qbpw, kvbkw, kvbvw

        def load_ow(nt):
            owt_a = sbow.tile([128, 8, 512], BF16, tag="owa", name="owt_a")
            owt_b = sbow.tile([128, 8, 512], BF16, tag="owb", name="owt_b")
            _dma_rows_to_3d(nc, owt_a[:],
                            ow.ap()[0:8 * 128, nt * 512:(nt + 1) * 512], 8)
            _dma_rows_to_3d(nc, owt_b[:],
                            ow.ap()[8 * 128:16 * 128, nt * 512:(nt + 1) * 512], 8)
            return owt_a, owt_b

        # ================= Phase A: fused a-proj + rmsnorm + kpe rope ======
        with ExitStack() as pa:
            sba = pa.enter_context(tc.tile_pool(name="sba", bufs=1))
            sbw = pa.enter_context(tc.tile_pool(name="sbw", bufs=3))
            sbt = pa.enter_context(tc.tile_pool(name="sbt", bufs=2))
            psa = pa.enter_context(tc.tile_pool(name="psa", bufs=1, space="PSUM"))

            xT_sb = sba.tile([128, CC_A, S_SH], BF16, tag="xT")
            for c0 in range(0, CC_A, 7):
                _dma_rows_to_3d(nc, xT_sb[:, c0:c0 + 7, :],
                                xT.ap()[c0 * 128:(c0 + 7) * 128, :], 7)
            acts = sba.tile([S_SH, CA], F32, tag="acts")

            pa_ps = [psa.tile([128, 512], F32, tag="a0", name="pa0"),
                     psa.tile([128, 512], F32, tag="a1", name="pa1"),
                     psa.tile([128, 512], F32, tag="a2", name="pa2"),
                     psa.tile([128, 512], F32, tag="a3", name="pa3"),
                     psa.tile([128, 64], F32, tag="a4", name="pa4")]
            for cc in range(CC_A):
                wt = sbw.tile([128, CA], BF16, tag="wa")
                nc.sync.dma_start(wt[:], wa.ap()[cc * 128:(cc + 1) * 128, :])
                for j, (d0, dn) in enumerate(NT_A):
                    nc.tensor.matmul(pa_ps[j][:, :dn], xT_sb[:, cc, :],
                                     wt[:, d0:d0 + dn],
                                     start=(cc == 0), stop=(cc == CC_A - 1))
            for j, (d0, dn) in enumerate(NT_A):
                nc.scalar.copy(acts[:, d0:d0 + dn], pa_ps[j][:, :dn])

            # rmsnorm factors for qc (cols 0:1536) and ckv (cols 1536:2048)
            sq = sba.tile([S_SH, CQ + CKV], F32, tag="sq")
            nc.vector.tensor_mul(sq[:], acts[:, 0:CQ + CKV], acts[:, 0:CQ + CKV])
            fq = sbt.tile([S_SH, 1], F32, tag="fq")
            fk = sbt.tile([S_SH, 1], F32, tag="fk")
            nc.vector.reduce_sum(fq[:], sq[:, 0:CQ], axis=mybir.AxisListType.X)
            nc.vector.reduce_sum(fk[:], sq[:, CQ:CQ + CKV], axis=mybir.AxisListType.X)
            nc.vector.tensor_scalar(fq[:], fq[:], 1.0 / CQ, EPS, OP.mult, OP.add)
            nc.vector.tensor_scalar(fk[:], fk[:], 1.0 / CKV, EPS, OP.mult, OP.add)
            nc.vector.reciprocal(fq[:], fq[:])
            nc.vector.reciprocal(fk[:], fk[:])
            nc.scalar.activation(fq[:], fq[:], AF.Sqrt)
            nc.scalar.activation(fk[:], fk[:], AF.Sqrt)
            nc.vector.tensor_scalar_mul(acts[:, 0:CQ], acts[:, 0:CQ], fq[:])
            nc.vector.tensor_scalar_mul(acts[:, CQ:CQ + CKV], acts[:, CQ:CQ + CKV], fk[:])

            # k_pe rope (natural [s, 64] layout), cols 2048:2112
            kp0 = CQ + CKV
            kv1 = sbt.tile([S_SH, D_ROPE], F32, tag="kv1")
            kv2 = sbt.tile([S_SH, D_ROPE], F32, tag="kv2")
            nc.vector.tensor_mul(kv1[:], acts[:, kp0:kp0 + 64], cos_s_sb[:])
            nc.vector.tensor_mul(kv2[:, 0:32], acts[:, kp0 + 32:kp0 + 64], sin_sg_sb[:, 0:32])
            nc.vector.tensor_mul(kv2[:, 32:64], acts[:, kp0:kp0 + 32], sin_sg_sb[:, 32:64])
            nc.vector.tensor_add(acts[:, kp0:kp0 + 64], kv1[:], kv2[:])

            # cast to bf16 and ship to the collective buffer (4KB rows)
            acts_bf = sba.tile([S_SH, CA], BF16, tag="acts_bf")
            nc.vector.tensor_copy(acts_bf[:], acts[:])
            nc.sync.dma_start(agi[:, :], acts_bf[:])

        # prefetch first two groups' weights + first ow tile before the AG
        # so the DMA queues aren't blocked behind AG-dependent transfers
        wq_pre = [load_group_weights(0), load_group_weights(1)]
        ow_pre = load_ow(0)

        nc.gpsimd.collective_compute(
            "AllGather", OP.bypass,
            replica_groups=[list(range(N_CORES))],
            ins=[agi.opt()], outs=[ago.opt()],
        )

        # ================= Phase B: per-head-group projections + attention ==
        with ExitStack() as pb:
            sbg = pb.enter_context(tc.tile_pool(name="sbg", bufs=1))
            sbh = pb.enter_context(tc.tile_pool(name="sbh", bufs=2))
            sbp = pb.enter_context(tc.tile_pool(name="sbp", bufs=1))
            sbv = pb.enter_context(tc.tile_pool(name="sbv", bufs=2))
            sbpt = pb.enter_context(tc.tile_pool(name="sbpt", bufs=6))
            sbs = pb.enter_context(tc.tile_pool(name="sbs", bufs=2))
            sbo = pb.enter_context(tc.tile_pool(name="sbo", bufs=3))
            ps_main = pb.enter_context(tc.tile_pool(name="ps_main", bufs=3, space="PSUM"))
            ps_o = pb.enter_context(tc.tile_pool(name="ps_o", bufs=2, space="PSUM"))
            ps_sm = pb.enter_context(tc.tile_pool(name="ps_sm", bufs=1, space="PSUM"))

            # stitch gathered activations into [K-chunk, seq] layout with
            # DMA-engine transposes (no PE/DVE cost)
            qct = []
            ckv = []
            for st in range(2):
                q_t = sbg.tile([128, CQ // 128, 512], BF16, tag=f"qct{st}",
                               name=f"qct{st}")
                k_t = sbg.tile([128, CKV // 128, 512], BF16, tag=f"ckv{st}",
                               name=f"ckv{st}")
                for c in range(CQ // 128):
                    nc.sync.dma_start_transpose(
                        q_t[:, c, :],
                        ago[st * 512:(st + 1) * 512, c * 128:(c + 1) * 128])
                for c in range(CKV // 128):
                    nc.sync.dma_start_transpose(
                        k_t[:, c, :],
                        ago[st * 512:(st + 1) * 512,
                            CQ + c * 128:CQ + (c + 1) * 128])
                qct.append(q_t)
                ckv.append(k_t)
            kpe2 = sbg.tile([128, S], BF16, tag="kpe2")
            for st in range(2):
                nc.sync.dma_start_transpose(
                    kpe2[0:64, st * 512:(st + 1) * 512],
                    ago[st * 512:(st + 1) * 512, CQ + CKV:CA])
                nc.sync.dma_start_transpose(
                    kpe2[64:128, st * 512:(st + 1) * 512],
                    ago[st * 512:(st + 1) * 512, CQ + CKV:CA])

            pending_norm = []

            def emit_normalize():
                while pending_norm:
                    h_idx, qt_, sums_, psum_o_ = pending_norm.pop(0)
                    sums_b = sbpt.tile([128, 512], BF16, tag="sums_b",
                                       name="sums_b")
                    nc.vector.tensor_copy(sums_b[:], sums_[:])
                    pss = ps_sm.tile([128, 512], F32, tag="pss", name="pss")
                    nc.tensor.matmul(pss[0:1, :], ones_col_sb[:], sums_b[:],
                                     start=True, stop=True)
                    rec = sbs.tile([1, 512], BF16, tag="rec", name="rec")
                    with nc.allow_low_precision(reason="softmax recip in bf16"):
                        nc.vector.reciprocal(rec[:], pss[0:1, :])
                    psb = ps_sm.tile([128, 512], F32, tag="pss", name="psb")
                    nc.tensor.matmul(psb[:], ones_row_sb[:], rec[:],
                                     start=True, stop=True)
                    bsb = sbs.tile([128, 512], F32, tag="bsb", name="bsb")
                    nc.vector.tensor_copy(bsb[:], psb[:])
                    nc.vector.tensor_mul(
                        outs_sb[:, h_idx, qt_ * 512:(qt_ + 1) * 512],
                        psum_o_[:], bsb[:])

            for g in range(N_GROUPS):
                h0 = g * G_HEADS
                if g < 2:
                    qbnw, qbpw, kvbkw, kvbvw = wq_pre[g]
                else:
                    qbnw, qbpw, kvbkw, kvbvw = load_group_weights(g)

                # --- projections; st-paired so each stationary is reused ---
                qTn = []
                kTn = []
                for i in range(G_HEADS):
                    qt_t = sbh.tile([128, S], BF16, tag=f"qTn{i}", name=f"qTn{i}")
                    p0 = ps_main.tile([128, 512], F32, tag="s", name="p0")
                    p1 = ps_main.tile([128, 512], F32, tag="s", name="p1")
                    for c in range(CQ // 128):
                        nc.tensor.matmul(p0[:], qbnw[:, c, i * 128:(i + 1) * 128],
                                         qct[0][:, c, :],
                                         start=(c == 0), stop=(c == CQ // 128 - 1))
                        nc.tensor.matmul(p1[:], qbnw[:, c, i * 128:(i + 1) * 128],
                                         qct[1][:, c, :],
                                         start=(c == 0), stop=(c == CQ // 128 - 1))
                    nc.scalar.copy(qt_t[:, 0:512], p0[:])
                    nc.scalar.copy(qt_t[:, 512:1024], p1[:])
                    qTn.append(qt_t)
                    if g > 0 or i > 0:
                        emit_normalize()
                for i in range(G_HEADS):
                    kt_t = sbh.tile([128, S], BF16, tag=f"kTn{i}", name=f"kTn{i}")
                    p0 = ps_main.tile([128, 512], F32, tag="s", name="p0")
                    p1 = ps_main.tile([128, 512], F32, tag="s", name="p1")
                    for c in range(CKV // 128):
                        nc.tensor.matmul(p0[:], kvbkw[:, c, i * 128:(i + 1) * 128],
                                         ckv[0][:, c, :],
                                         start=(c == 0), stop=(c == CKV // 128 - 1))
                        nc.tensor.matmul(p1[:], kvbkw[:, c, i * 128:(i + 1) * 128],
                                         ckv[1][:, c, :],
                                         start=(c == 0), stop=(c == CKV // 128 - 1))
                    nc.scalar.copy(kt_t[:, 0:512], p0[:])
                    nc.scalar.copy(kt_t[:, 512:1024], p1[:])
                    kTn.append(kt_t)

                qp_raw = sbp.tile([128, S], F32, tag="qp_raw")
                p0 = ps_main.tile([128, 512], F32, tag="s", name="p0")
                p1 = ps_main.tile([128, 512], F32, tag="s", name="p1")
                for c in range(CQ // 128):
                    nc.tensor.matmul(p0[:], qbpw[:, c, :], qct[0][:, c, :],
                                     start=(c == 0), stop=(c == CQ // 128 - 1))
                    nc.tensor.matmul(p1[:], qbpw[:, c, :], qct[1][:, c, :],
                                     start=(c == 0), stop=(c == CQ // 128 - 1))
                nc.scalar.copy(qp_raw[:, 0:512], p0[:])
                nc.scalar.copy(qp_raw[:, 512:1024], p1[:])
                # rope on the head-pair tile: rows [0:64]=head h0, [64:128]=h0+1
                qTp = sbh.tile([128, S], BF16, tag="qTp")
                rm = sbp.tile([128, S], F32, tag="ropem")
                rs = sbp.tile([128, S], F32, tag="ropes")
                nc.vector.tensor_mul(rm[:], qp_raw[:], cos2t_sb[:])
                for b in range(4):
                    r0 = b * 32
                    r1 = r0 + 32 if b % 2 == 0 else r0 - 32
                    nc.vector.tensor_copy(rs[r0:r0 + 32, :], qp_raw[r1:r1 + 32, :])
                nc.vector.tensor_mul(rs[:], rs[:], sin2tg_sb[:])
                nc.vector.tensor_add(qTp[:], rm[:], rs[:])

                v_g = sbv.tile([128, 8, G_HEADS * 128], BF16, tag="v_g")
                for sc in range(8):
                    st = sc // 4
                    psum = ps_main.tile([128, 512], F32, tag="s", name="psum")
                    nn = G_HEADS * 128
                    for c in range(CKV // 128):
                        nc.tensor.matmul(
                            psum[:, :nn],
                            ckv[st][:, c, (sc % 4) * 128:(sc % 4 + 1) * 128],
                            kvbvw[:, c, :],
                            start=(c == 0), stop=(c == CKV // 128 - 1))
                    nc.vector.tensor_copy(v_g[:, sc, :], psum[:, :nn])

                # --- attention: heads interleaved, AV skewed one kc behind ---
                for qt in range(2):
                    kmax = 4 * (qt + 1)
                    sums = [sbs.tile([128, 512], F32, tag=f"sums{i}", name=f"sums{i}")
                            for i in range(G_HEADS)]
                    psum_o = [ps_o.tile([128, 512], F32, tag=f"o{i}", name=f"po{i}")
                              for i in range(G_HEADS)]
                    pt = {}
                    for kc in range(kmax):
                        for i in range(G_HEADS):
                            ps = ps_main.tile([128, 512], F32, tag="s", name="ps")
                            nc.tensor.matmul(ps[:], kTn[i][:, kc * 128:(kc + 1) * 128],
                                             qTn[i][:, qt * 512:(qt + 1) * 512],
                                             start=True, stop=False)
                            b = i * 64
                            nc.tensor.matmul(ps[:], kpe2[b:b + 64, kc * 128:(kc + 1) * 128],
                                             qTp[b:b + 64, qt * 512:(qt + 1) * 512],
                                             start=False, stop=True)
                            p = sbpt.tile([128, 512], BF16, tag="pt", name="p")
                            nc.scalar.activation(p[:], ps[:], AF.Exp, scale=SCALE)
                            if kc >= 4 * qt:
                                nc.vector.tensor_mul(p[:], p[:],
                                                     masks_sb[:, kc - 4 * qt, :])
                            if kc == 0:
                                nc.vector.tensor_copy(sums[i][:], p[:])
                            else:
                                nc.vector.tensor_add(sums[i][:], sums[i][:], p[:])
                            pt[(i, kc)] = p
                        if kc > 0:
                            for i in range(G_HEADS):
                                nc.tensor.matmul(psum_o[i][:],
                                                 v_g[:, kc - 1, i * 128:(i + 1) * 128],
                                                 pt[(i, kc - 1)][:],
                                                 start=(kc == 1), stop=False)
                        if qt == 1 and kc == 1:
                            emit_normalize()   # previous qt's softmax tail
                    for i in range(G_HEADS):
                        nc.tensor.matmul(psum_o[i][:],
                                         v_g[:, kmax - 1, i * 128:(i + 1) * 128],
                                         pt[(i, kmax - 1)][:],
                                         start=(kmax == 1), stop=True)
                    for i in range(G_HEADS):
                        pending_norm.append((h0 + i, qt, sums[i], psum_o[i]))

            emit_normalize()

            # ========= Phase C: partial output projection, out^T layout =====
            # out^T[hid, s] = sum_hc ow[hc]^T @ outs[hc]; each ow stationary
            # slice serves both 512-wide s-blocks before the next LDWEIGHTS.
            for nt in range(HID // 512):
                owt_a, owt_b = ow_pre if nt == 0 else load_ow(nt)
                for ntl in range(4):
                    pA = ps_main.tile([128, 512], F32, tag="s", name="pA")
                    pB = ps_main.tile([128, 512], F32, tag="s", name="pB")
                    for hc in range(HG):
                        owt = owt_a if hc < 8 else owt_b
                        lhs = owt[:, hc % 8, ntl * 128:(ntl + 1) * 128]
                        nc.tensor.matmul(pA[:], lhs, outs_sb[:, hc, 0:512],
                                         start=(hc == 0), stop=(hc == HG - 1))
                        nc.tensor.matmul(pB[:], lhs, outs_sb[:, hc, 512:1024],
                                         start=(hc == 0), stop=(hc == HG - 1))
                    for half, pp in ((0, pA), (1, pB)):
                        osb = sbo.tile([128, 512], BF16, tag="osb", name="osb")
                        nc.scalar.copy(osb[:], pp[:])
                        nc.sync.dma_start(
                            outT.ap()[nt * 512 + ntl * 128:nt * 512 + (ntl + 1) * 128,
                                      half * 512:(half + 1) * 512], osb[:])

    nc.compile()
    return nc


def _host_inputs(hidden_states, position_ids, q_a_weight, q_a_layernorm_weight,
                 q_b_weight, kv_a_weight, kv_a_layernorm_weight, kv_b_weight,
                 o_weight):
    bf16 = ml_dtypes.bfloat16
    x = np.asarray(hidden_states, np.float32).reshape(S, HID)
    pos = np.asarray(position_ids, np.float64).reshape(S)
    q_a_w = np.asarray(q_a_weight, np.float32)
    q_ln = np.asarray(q_a_layernorm_weight, np.float32)
    q_b_w = np.asarray(q_b_weight, np.float32)
    kv_a_w = np.asarray(kv_a_weight, np.float32)
    kv_ln = np.asarray(kv_a_layernorm_weight, np.float32)
    kv_b_w = np.asarray(kv_b_weight, np.float32)
    o_w = np.asarray(o_weight, np.float32)

    wa = np.concatenate([q_a_w, kv_a_w], axis=1).astype(bf16)  # [HID, 2112]
    xT = np.ascontiguousarray(x.T).astype(bf16)                # [HID, S]

    # fold the rms-norm weights into the b-projections
    qb = (q_ln[:, None] * q_b_w).reshape(CQ, H, D_Q)
    kvb = (kv_ln[:, None] * kv_b_w).reshape(CKV, H, D_NOPE + D_V)

    # rope tables
    inv_freq = 1.0 / (10000.0 ** (np.arange(0, D_ROPE, 2, dtype=np.float64) / D_ROPE))
    freqs = pos[:, None] * inv_freq[None, :]                # [S, 32]
    emb = np.concatenate([freqs, freqs], axis=-1)           # [S, 64]
    cos = np.cos(emb).astype(np.float32)
    sin = np.sin(emb).astype(np.float32)
    sin_sg = np.concatenate([-sin[:, :32], sin[:, 32:]], axis=1)  # [S, 64]
    cosT = np.ascontiguousarray(cos.T)                      # [64, S]
    sinT_sg = np.ascontiguousarray(sin_sg.T)                # [64, S]
    cos2t = np.concatenate([cosT, cosT], axis=0)            # [128, S]
    sin2tg = np.concatenate([sinT_sg, sinT_sg], axis=0)     # [128, S]

    # causal masks for the 4 diagonal offsets
    masks = np.zeros((4, 128, 512), np.float32)
    i = np.arange(128)[:, None]
    j = np.arange(512)[None, :]
    for m in range(4):
        masks[m] = ((i + m * 128) <= j).astype(np.float32)
    masks = masks.reshape(512, 512).astype(bf16)

    ones_col = np.ones((128, 1), bf16)
    ones_row = np.ones((1, 128), bf16)

    in_maps = []
    for c in range(N_CORES):
        hs = slice(c * HG, (c + 1) * HG)
        in_maps.append({
            "xT": np.ascontiguousarray(xT[:, c * S_SH:(c + 1) * S_SH]),
            "wa": wa,
            "qbn": np.ascontiguousarray(
                qb[:, hs, :D_NOPE].reshape(CQ, HG * D_NOPE)).astype(bf16),
            "qbp": np.ascontiguousarray(
                qb[:, hs, D_NOPE:].reshape(CQ, HG * D_ROPE)).astype(bf16),
            "kvbk": np.ascontiguousarray(
                kvb[:, hs, :D_NOPE].reshape(CKV, HG * D_NOPE)).astype(bf16),
            "kvbv": np.ascontiguousarray(
                kvb[:, hs, D_NOPE:].reshape(CKV, HG * D_V)).astype(bf16),
            "ow": np.ascontiguousarray(
                o_w[c * HG * D_V:(c + 1) * HG * D_V, :]).astype(bf16),
            "cos_s": np.ascontiguousarray(cos[c * S_SH:(c + 1) * S_SH, :]),
            "sin_sg": np.ascontiguousarray(sin_sg[c * S_SH:(c + 1) * S_SH, :]),
            "cos2t": cos2t,
            "sin2tg": sin2tg,
            "masks": masks,
            "ones_col": ones_col,
            "ones_row": ones_row,
        })
    return in_maps


def kernel(**inputs):
    global LAST_EXEC_NS
    trace = bool(inputs.pop("_trace", False))
    in_maps = _host_inputs(**inputs)
    if "nc" not in _CACHE:
        _CACHE["nc"] = _build_nc()
    nc = _CACHE["nc"]
    res = bass_utils.run_bass_kernel_spmd(
        nc, in_maps, core_ids=list(range(N_CORES)), trace=trace)
    LAST_EXEC_NS = res.exec_time_ns
    total = np.zeros((HID, S), np.float64)
    for c in range(N_CORES):
        total += res.results[c]["outT"].astype(np.float64)
    return np.ascontiguousarray(total.T).astype(np.float32).reshape(1, 1, S, HID)


# revision 15
# speedup vs baseline: 1.1548x; 1.1548x over previous
"""DeepseekV3 MLA attention prefill (S=1024, H=128 heads, HID=7168) on 8 TRN2
NeuronCores.

Sharding: tensor-parallel over heads (16 heads/core); the low-rank input
projections (q_a / kv_a) are sequence-sharded (128 rows/core) and exchanged
with one AllGather of the rms-normed activations (natural [s, col] layout so
the collective moves 4KB rows). Each core emits a partial output projection
(contraction over its own 16 heads, transposed [HID, S] layout so each ow
stationary tile serves two matmuls); the host sums the 8 partials.

All matmul operands are bf16 (weights pre-cast on host, activations cast at
the psum->SBUF copy); softmax/rmsnorm math stays f32. The attention inner
loop is software-pipelined: AV matmuls run one kc-step behind the score
matmuls and the softmax normalization matmuls are deferred into the next
qt/group's instruction stream so exp/mask/reduce latency never stalls the
in-order PE queue. Post-AllGather stitching uses DMA-engine transposes.
"""
import math
import numpy as np
import ml_dtypes

import concourse.bass as bass
import concourse.mybir as mybir
import concourse.bacc as bacc
import concourse.tile as tile
import concourse.bass_utils as bass_utils
from concourse.masks import make_identity
from contextlib import ExitStack

F32 = mybir.dt.float32
BF16 = mybir.dt.bfloat16
AF = mybir.ActivationFunctionType
OP = mybir.AluOpType

N_CORES = 8
S = 1024
HID = 7168
H = 128
HG = H // N_CORES          # 16 heads per core
D_NOPE = 128
D_ROPE = 64
D_Q = D_NOPE + D_ROPE      # 192
D_V = 128
CQ = 1536                  # q lora rank
CKV = 512                  # kv lora rank
CA = CQ + CKV + D_ROPE     # 2112 fused a-proj cols
S_SH = S // N_CORES        # 128 sequence rows per core
CC_A = HID // 128          # 56 contraction chunks for a-proj
NT_A = [(0, 512), (512, 512), (1024, 512), (1536, 512), (2048, 64)]
SCALE = 1.0 / math.sqrt(D_Q)
EPS = 1e-6
G_HEADS = 2                # heads per group
N_GROUPS = HG // G_HEADS   # 8 groups
LAST_EXEC_NS = None

_CACHE = {}


def _dma_rows_to_3d(nc, dst, src_ap, n_chunks, p=128):
    """dst [p, n_chunks, w] <- src rows laid out as (chunk, p)."""
    try:
        nc.sync.dma_start(dst, src_ap.rearrange("(c p) s -> p c s", p=p))
    except Exception:
        for c in range(n_chunks):
            nc.sync.dma_start(dst[:, c, :], src_ap[c * p:(c + 1) * p, :])


def _build_nc():
    nc = bacc.Bacc("TRN2", target_bir_lowering=False, debug=False,
                   num_devices=N_CORES)

    xT = nc.dram_tensor("xT", [HID, S_SH], BF16, kind="ExternalInput")
    wa = nc.dram_tensor("wa", [HID, CA], BF16, kind="ExternalInput")
    qbn = nc.dram_tensor("qbn", [CQ, HG * D_NOPE], BF16, kind="ExternalInput")
    qbp = nc.dram_tensor("qbp", [CQ, HG * D_ROPE], BF16, kind="ExternalInput")
    kvbk = nc.dram_tensor("kvbk", [CKV, HG * D_NOPE], BF16, kind="ExternalInput")
    kvbv = nc.dram_tensor("kvbv", [CKV, HG * D_V], BF16, kind="ExternalInput")
    ow = nc.dram_tensor("ow", [HG * D_V, HID], BF16, kind="ExternalInput")
    cos_s = nc.dram_tensor("cos_s", [S_SH, D_ROPE], F32, kind="ExternalInput")
    sin_sg = nc.dram_tensor("sin_sg", [S_SH, D_ROPE], F32, kind="ExternalInput")
    cos2t = nc.dram_tensor("cos2t", [128, S], F32, kind="ExternalInput")
    sin2tg = nc.dram_tensor("sin2tg", [128, S], F32, kind="ExternalInput")
    masks = nc.dram_tensor("masks", [512, 512], BF16, kind="ExternalInput")
    ones_col = nc.dram_tensor("ones_col", [128, 1], BF16, kind="ExternalInput")
    ones_row = nc.dram_tensor("ones_row", [1, 128], BF16, kind="ExternalInput")
    outT = nc.dram_tensor("outT", [HID, S], BF16, kind="ExternalOutput")

    with tile.TileContext(nc) as tc, ExitStack() as top:
        const = top.enter_context(tc.tile_pool(name="const", bufs=1))
        dram = top.enter_context(tc.tile_pool(name="dram", bufs=1, space="DRAM"))
        outsp = top.enter_context(tc.tile_pool(name="outsp", bufs=1))
        # phase B/C weight + staging pools opened at top level so their
        # prefetch DMAs can be emitted before the AllGather
        sbwq = top.enter_context(tc.tile_pool(name="sbwq", bufs=2))
        sbow = top.enter_context(tc.tile_pool(name="sbow", bufs=2))

        # ---- constants in SBUF ----
        ident = const.tile([128, 128], F32, tag="ident")
        make_identity(nc, ident[:])
        masks_sb = const.tile([128, 4, 512], BF16, tag="masks")
        _dma_rows_to_3d(nc, masks_sb[:], masks.ap(), 4)
        cos_s_sb = const.tile([S_SH, D_ROPE], F32, tag="coss")
        sin_sg_sb = const.tile([S_SH, D_ROPE], F32, tag="sinsg")
        nc.sync.dma_start(cos_s_sb[:], cos_s.ap())
        nc.sync.dma_start(sin_sg_sb[:], sin_sg.ap())
        cos2t_sb = const.tile([128, S], F32, tag="cos2t")
        sin2tg_sb = const.tile([128, S], F32, tag="sin2tg")
        nc.sync.dma_start(cos2t_sb[:], cos2t.ap())
        nc.sync.dma_start(sin2tg_sb[:], sin2tg.ap())
        ones_col_sb = const.tile([128, 1], BF16, tag="onesc")
        ones_row_sb = const.tile([1, 128], BF16, tag="onesr")
        nc.sync.dma_start(ones_col_sb[:], ones_col.ap())
        nc.sync.dma_start(ones_row_sb[:], ones_row.ap())

        agi = dram.tile([CA, S_SH], BF16, tag="agi")
        ago = dram.tile([CA * N_CORES, S_SH], BF16, tag="ago", addr_space="Shared")

        # all 16 heads' attention outputs live in SBUF [dv=128, head, s]
        outs_sb = outsp.tile([128, HG, S], BF16, tag="outs")

        def load_group_weights(g):
            h0 = g * G_HEADS
            qbnw = sbwq.tile([128, CQ // 128, G_HEADS * 128], BF16,
                             tag="qbnw", name="qbnw")
            qbpw = sbwq.tile([128, CQ // 128, G_HEADS * 64], BF16,
                             tag="qbpw", name="qbpw")
            kvbkw = sbwq.tile([128, CKV // 128, G_HEADS * 128], BF16,
                              tag="kvbkw", name="kvbkw")
            kvbvw = sbwq.tile([128, CKV // 128, G_HEADS * 128], BF16,
                              tag="kvbvw", name="kvbvw")
            _dma_rows_to_3d(nc, qbnw[:],
                            qbn.ap()[:, h0 * 128:(h0 + G_HEADS) * 128], CQ // 128)
            _dma_rows_to_3d(nc, qbpw[:],
                            qbp.ap()[:, h0 * 64:(h0 + G_HEADS) * 64], CQ // 128)
            _dma_rows_to_3d(nc, kvbkw[:],
                            kvbk.ap()[:, h0 * 128:(h0 + G_HEADS) * 128], CKV // 128)
            _dma_rows_to_3d(nc, kvbvw[:],
                            kvbv.ap()[:, h0 * 128:(h0 + G_HEADS) * 128], CKV // 128)
            return qbnw, qbpw, kvbkw, kvbvw

        def load_ow(nt):
            owt_a = sbow.tile([128, 8, 512], BF16, tag="owa", name="owt_a")
            owt_b = sbow.tile([128, 8, 512], BF16, tag="owb", name="owt_b")
            _dma_rows_to_3d(nc, owt_a[:],
                            ow.ap()[0:8 * 128, nt * 512:(nt + 1) * 512], 8)
            _dma_rows_to_3d(nc, owt_b[:],
                            ow.ap()[8 * 128:16 * 128, nt * 512:(nt + 1) * 512], 8)
            return owt_a, owt_b

        # ================= Phase A: fused a-proj + rmsnorm + kpe rope ======
        with ExitStack() as pa:
            sba = pa.enter_context(tc.tile_pool(name="sba", bufs=1))
            sbw = pa.enter_context(tc.tile_pool(name="sbw", bufs=3))
            sbt = pa.enter_context(tc.tile_pool(name="sbt", bufs=2))
            psa = pa.enter_context(tc.tile_pool(name="psa", bufs=1, space="PSUM"))

            xT_sb = sba.tile([128, CC_A, S_SH], BF16, tag="xT")
            for c0 in range(0, CC_A, 7):
                _dma_rows_to_3d(nc, xT_sb[:, c0:c0 + 7, :],
                                xT.ap()[c0 * 128:(c0 + 7) * 128, :], 7)
            acts = sba.tile([S_SH, CA], F32, tag="acts")

            pa_ps = [psa.tile([128, 512], F32, tag="a0", name="pa0"),
                     psa.tile([128, 512], F32, tag="a1", name="pa1"),
                     psa.tile([128, 512], F32, tag="a2", name="pa2"),
                     psa.tile([128, 512], F32, tag="a3", name="pa3"),
                     psa.tile([128, 64], F32, tag="a4", name="pa4")]
            for cc in range(CC_A):
                wt = sbw.tile([128, CA], BF16, tag="wa")
                nc.sync.dma_start(wt[:], wa.ap()[cc * 128:(cc + 1) * 128, :])
                for j, (d0, dn) in enumerate(NT_A):
                    nc.tensor.matmul(pa_ps[j][:, :dn], xT_sb[:, cc, :],
                                     wt[:, d0:d0 + dn],
                                     start=(cc == 0), stop=(cc == CC_A - 1))
            for j, (d0, dn) in enumerate(NT_A):
                nc.scalar.copy(acts[:, d0:d0 + dn], pa_ps[j][:, :dn])

            # rmsnorm factors for qc (cols 0:1536) and ckv (cols 1536:2048)
            sq = sba.tile([S_SH, CQ + CKV], F32, tag="sq")
            nc.vector.tensor_mul(sq[:], acts[:, 0:CQ + CKV], acts[:, 0:CQ + CKV])
            fq = sbt.tile([S_SH, 1], F32, tag="fq")
            fk = sbt.tile([S_SH, 1], F32, tag="fk")
            nc.vector.reduce_sum(fq[:], sq[:, 0:CQ], axis=mybir.AxisListType.X)
            nc.vector.reduce_sum(fk[:], sq[:, CQ:CQ + CKV], axis=mybir.AxisListType.X)
            nc.vector.tensor_scalar(fq[:], fq[:], 1.0 / CQ, EPS, OP.mult, OP.add)
            nc.vector.tensor_scalar(fk[:], fk[:], 1.0 / CKV, EPS, OP.mult, OP.add)
            nc.vector.reciprocal(fq[:], fq[:])
            nc.vector.reciprocal(fk[:], fk[:])
            nc.scalar.activation(fq[:], fq[:], AF.Sqrt)
            nc.scalar.activation(fk[:], fk[:], AF.Sqrt)
            nc.vector.tensor_scalar_mul(acts[:, 0:CQ], acts[:, 0:CQ], fq[:])
            nc.vector.tensor_scalar_mul(acts[:, CQ:CQ + CKV], acts[:, CQ:CQ + CKV], fk[:])

            # k_pe rope (natural [s, 64] layout), cols 2048:2112
            kp0 = CQ + CKV
            kv1 = sbt.tile([S_SH, D_ROPE], F32, tag="kv1")
            kv2 = sbt.tile([S_SH, D_ROPE], F32, tag="kv2")
            nc.vector.tensor_mul(kv1[:], acts[:, kp0:kp0 + 64], cos_s_sb[:])
            nc.vector.tensor_mul(kv2[:, 0:32], acts[:, kp0 + 32:kp0 + 64], sin_sg_sb[:, 0:32])
            nc.vector.tensor_mul(kv2[:, 32:64], acts[:, kp0:kp0 + 32], sin_sg_sb[:, 32:64])
            nc.vector.tensor_add(acts[:, kp0:kp0 + 64], kv1[:], kv2[:])

            # transpose all 17 chunks -> bounce [2112, 128] in bf16
            bT = sba.tile([128, 17 * 128], BF16, tag="bT")
            for t in range(17):
                w = 128 if t < 16 else 64
                pt_ps = psa.tile([128, 128], F32, tag="tp", name="pt_ps")
                nc.tensor.transpose(pt_ps[:w, 0:128], acts[:, t * 128:t * 128 + w], ident[:])
                nc.scalar.copy(bT[:w, t * 128:(t + 1) * 128], pt_ps[:w, 0:128])
                nc.sync.dma_start(agi[t * 128:t * 128 + w, :], bT[:w, t * 128:(t + 1) * 128])

        # prefetch first two groups' weights + first ow tile before the AG
        # so the DMA queues aren't blocked behind AG-dependent transfers
        wq_pre = [load_group_weights(0), load_group_weights(1)]
        ow_pre = load_ow(0)

        nc.gpsimd.collective_compute(
            "AllGather", OP.bypass,
            replica_groups=[list(range(N_CORES))],
            ins=[agi.opt()], outs=[ago.opt()],
        )

        # ================= Phase B: per-head-group projections + attention ==
        with ExitStack() as pb:
            sbg = pb.enter_context(tc.tile_pool(name="sbg", bufs=1))
            sbh = pb.enter_context(tc.tile_pool(name="sbh", bufs=2))
            sbp = pb.enter_context(tc.tile_pool(name="sbp", bufs=1))
            sbv = pb.enter_context(tc.tile_pool(name="sbv", bufs=2))
            sbpt = pb.enter_context(tc.tile_pool(name="sbpt", bufs=6))
            sbs = pb.enter_context(tc.tile_pool(name="sbs", bufs=2))
            sbo = pb.enter_context(tc.tile_pool(name="sbo", bufs=3))
            ps_main = pb.enter_context(tc.tile_pool(name="ps_main", bufs=3, space="PSUM"))
            ps_o = pb.enter_context(tc.tile_pool(name="ps_o", bufs=2, space="PSUM"))
            ps_sm = pb.enter_context(tc.tile_pool(name="ps_sm", bufs=1, space="PSUM"))

            # stitch gathered activations, per 512-wide s-tile
            qct = []
            ckv = []
            for st in range(2):
                q_t = sbg.tile([128, CQ // 128, 512], BF16, tag=f"qct{st}",
                               name=f"qct{st}")
                k_t = sbg.tile([128, CKV // 128, 512], BF16, tag=f"ckv{st}",
                               name=f"ckv{st}")
                for r in range(4):
                    core = st * 4 + r
                    base = core * CA
                    _dma_rows_to_3d(nc, q_t[:, :, r * 128:(r + 1) * 128],
                                    ago[base:base + CQ, :], CQ // 128)
                    _dma_rows_to_3d(nc, k_t[:, :, r * 128:(r + 1) * 128],
                                    ago[base + CQ:base + CQ + CKV, :], CKV // 128)
                qct.append(q_t)
                ckv.append(k_t)
            kpe2 = sbg.tile([128, S], BF16, tag="kpe2")
            for core in range(N_CORES):
                base = core * CA + CQ + CKV
                nc.sync.dma_start(kpe2[0:64, core * 128:(core + 1) * 128],
                                  ago[base:base + 64, :])
                nc.sync.dma_start(kpe2[64:128, core * 128:(core + 1) * 128],
                                  ago[base:base + 64, :])

            pending_norm = []

            def emit_normalize():
                while pending_norm:
                    h_idx, qt_, sums_, psum_o_ = pending_norm.pop(0)
                    sums_b = sbpt.tile([128, 512], BF16, tag="sums_b",
                                       name="sums_b")
                    nc.vector.tensor_copy(sums_b[:], sums_[:])
                    pss = ps_sm.tile([128, 512], F32, tag="pss", name="pss")
                    nc.tensor.matmul(pss[0:1, :], ones_col_sb[:], sums_b[:],
                                     start=True, stop=True)
                    rec = sbs.tile([1, 512], BF16, tag="rec", name="rec")
                    with nc.allow_low_precision(reason="softmax recip in bf16"):
                        nc.vector.reciprocal(rec[:], pss[0:1, :])
                    psb = ps_sm.tile([128, 512], F32, tag="pss", name="psb")
                    nc.tensor.matmul(psb[:], ones_row_sb[:], rec[:],
                                     start=True, stop=True)
                    bsb = sbs.tile([128, 512], F32, tag="bsb", name="bsb")
                    nc.vector.tensor_copy(bsb[:], psb[:])
                    nc.vector.tensor_mul(
                        outs_sb[:, h_idx, qt_ * 512:(qt_ + 1) * 512],
                        psum_o_[:], bsb[:])

            for g in range(N_GROUPS):
                h0 = g * G_HEADS
                if g < 2:
                    qbnw, qbpw, kvbkw, kvbvw = wq_pre[g]
                else:
                    qbnw, qbpw, kvbkw, kvbvw = load_group_weights(g)

                # --- projections; st-paired so each stationary is reused ---
                qTn = []
                kTn = []
                for i in range(G_HEADS):
                    qt_t = sbh.tile([128, S], BF16, tag=f"qTn{i}", name=f"qTn{i}")
                    p0 = ps_main.tile([128, 512], F32, tag="s", name="p0")
                    p1 = ps_main.tile([128, 512], F32, tag="s", name="p1")
                    for c in range(CQ // 128):
                        nc.tensor.matmul(p0[:], qbnw[:, c, i * 128:(i + 1) * 128],
                                         qct[0][:, c, :],
                                         start=(c == 0), stop=(c == CQ // 128 - 1))
                        nc.tensor.matmul(p1[:], qbnw[:, c, i * 128:(i + 1) * 128],
                                         qct[1][:, c, :],
                                         start=(c == 0), stop=(c == CQ // 128 - 1))
                    nc.scalar.copy(qt_t[:, 0:512], p0[:])
                    nc.scalar.copy(qt_t[:, 512:1024], p1[:])
                    qTn.append(qt_t)
                    if g > 0 or i > 0:
                        emit_normalize()
                for i in range(G_HEADS):
                    kt_t = sbh.tile([128, S], BF16, tag=f"kTn{i}", name=f"kTn{i}")
                    p0 = ps_main.tile([128, 512], F32, tag="s", name="p0")
                    p1 = ps_main.tile([128, 512], F32, tag="s", name="p1")
                    for c in range(CKV // 128):
                        nc.tensor.matmul(p0[:], kvbkw[:, c, i * 128:(i + 1) * 128],
                                         ckv[0][:, c, :],
                                         start=(c == 0), stop=(c == CKV // 128 - 1))
                        nc.tensor.matmul(p1[:], kvbkw[:, c, i * 128:(i + 1) * 128],
                                         ckv[1][:, c, :],
                                         start=(c == 0), stop=(c == CKV // 128 - 1))
                    nc.scalar.copy(kt_t[:, 0:512], p0[:])
                    nc.scalar.copy(kt_t[:, 512:1024], p1[:])
                    kTn.append(kt_t)

                qp_raw = sbp.tile([128, S], F32, tag="qp_raw")
                p0 = ps_main.tile([128, 512], F32, tag="s", name="p0")
                p1 = ps_main.tile([128, 512], F32, tag="s", name="p1")
                for c in range(CQ // 128):
                    nc.tensor.matmul(p0[:], qbpw[:, c, :], qct[0][:, c, :],
                                     start=(c == 0), stop=(c == CQ // 128 - 1))
                    nc.tensor.matmul(p1[:], qbpw[:, c, :], qct[1][:, c, :],
                                     start=(c == 0), stop=(c == CQ // 128 - 1))
                nc.scalar.copy(qp_raw[:, 0:512], p0[:])
                nc.scalar.copy(qp_raw[:, 512:1024], p1[:])
                # rope on the head-pair tile: rows [0:64]=head h0, [64:128]=h0+1
                qTp = sbh.tile([128, S], BF16, tag="qTp")
                rm = sbp.tile([128, S], F32, tag="ropem")
                rs = sbp.tile([128, S], F32, tag="ropes")
                for hh in range(2):
                    sl = slice(hh * 512, (hh + 1) * 512)
                    nc.vector.tensor_mul(rm[:, sl], qp_raw[:, sl], cos2t_sb[:, sl])
                    for b in range(4):
                        r0 = b * 32
                        r1 = r0 + 32 if b % 2 == 0 else r0 - 32
                        nc.vector.tensor_copy(rs[r0:r0 + 32, sl], qp_raw[r1:r1 + 32, sl])
                    nc.vector.tensor_mul(rs[:, sl], rs[:, sl], sin2tg_sb[:, sl])
                    nc.vector.tensor_add(qTp[:, sl], rm[:, sl], rs[:, sl])

                v_g = sbv.tile([128, 8, G_HEADS * 128], BF16, tag="v_g")
                for sc in range(8):
                    st = sc // 4
                    psum = ps_main.tile([128, 512], F32, tag="s", name="psum")
                    nn = G_HEADS * 128
                    for c in range(CKV // 128):
                        nc.tensor.matmul(
                            psum[:, :nn],
                            ckv[st][:, c, (sc % 4) * 128:(sc % 4 + 1) * 128],
                            kvbvw[:, c, :],
                            start=(c == 0), stop=(c == CKV // 128 - 1))
                    nc.scalar.copy(v_g[:, sc, :], psum[:, :nn])

                # --- attention: heads interleaved, AV skewed one kc behind ---
                for qt in range(2):
                    kmax = 4 * (qt + 1)
                    sums = [sbs.tile([128, 512], F32, tag=f"sums{i}", name=f"sums{i}")
                            for i in range(G_HEADS)]
                    psum_o = [ps_o.tile([128, 512], F32, tag=f"o{i}", name=f"po{i}")
                              for i in range(G_HEADS)]
                    pt = {}
                    for kc in range(kmax):
                        for i in range(G_HEADS):
                            ps = ps_main.tile([128, 512], F32, tag="s", name="ps")
                            nc.tensor.matmul(ps[:], kTn[i][:, kc * 128:(kc + 1) * 128],
                                             qTn[i][:, qt * 512:(qt + 1) * 512],
                                             start=True, stop=False)
                            b = i * 64
                            nc.tensor.matmul(ps[:], kpe2[b:b + 64, kc * 128:(kc + 1) * 128],
                                             qTp[b:b + 64, qt * 512:(qt + 1) * 512],
                                             start=False, stop=True)
                            p = sbpt.tile([128, 512], BF16, tag="pt", name="p")
                            nc.scalar.activation(p[:], ps[:], AF.Exp, scale=SCALE)
                            if kc >= 4 * qt:
                                nc.vector.tensor_mul(p[:], p[:],
                                                     masks_sb[:, kc - 4 * qt, :])
                            if kc == 0:
                                nc.vector.tensor_copy(sums[i][:], p[:])
                            else:
                                nc.vector.tensor_add(sums[i][:], sums[i][:], p[:])
                            pt[(i, kc)] = p
                        if kc > 0:
                            for i in range(G_HEADS):
                                nc.tensor.matmul(psum_o[i][:],
                                                 v_g[:, kc - 1, i * 128:(i + 1) * 128],
                                                 pt[(i, kc - 1)][:],
                                                 start=(kc == 1), stop=False)
                        if qt == 1 and kc == 1:
                            emit_normalize()   # previous qt's softmax tail
                    for i in range(G_HEADS):
                        nc.tensor.matmul(psum_o[i][:],
                                         v_g[:, kmax - 1, i * 128:(i + 1) * 128],
                                         pt[(i, kmax - 1)][:],
                                         start=(kmax == 1), stop=True)
                    for i in range(G_HEADS):
                        pending_norm.append((h0 + i, qt, sums[i], psum_o[i]))

            emit_normalize()

            # ========= Phase C: partial output projection, out^T layout =====
            # out^T[hid, s] = sum_hc ow[hc]^T @ outs[hc]; each ow stationary
            # slice serves both 512-wide s-blocks before the next LDWEIGHTS.
            for nt in range(HID // 512):
                owt_a, owt_b = ow_pre if nt == 0 else load_ow(nt)
                for ntl in range(4):
                    pA = ps_main.tile([128, 512], F32, tag="s", name="pA")
                    pB = ps_main.tile([128, 512], F32, tag="s", name="pB")
                    for hc in range(HG):
                        owt = owt_a if hc < 8 else owt_b
                        lhs = owt[:, hc % 8, ntl * 128:(ntl + 1) * 128]
                        nc.tensor.matmul(pA[:], lhs, outs_sb[:, hc, 0:512],
                                         start=(hc == 0), stop=(hc == HG - 1))
                        nc.tensor.matmul(pB[:], lhs, outs_sb[:, hc, 512:1024],
                                         start=(hc == 0), stop=(hc == HG - 1))
                    for half, pp in ((0, pA), (1, pB)):
                        osb = sbo.tile([128, 512], BF16, tag="osb", name="osb")
                        nc.scalar.copy(osb[:], pp[:])
                        nc.sync.dma_start(
                            outT.ap()[nt * 512 + ntl * 128:nt * 512 + (ntl + 1) * 128,
                                      half * 512:(half + 1) * 512], osb[:])

    nc.compile()
    return nc


def _host_inputs(hidden_states, position_ids, q_a_weight, q_a_layernorm_weight,
                 q_b_weight, kv_a_weight, kv_a_layernorm_weight, kv_b_weight,
                 o_weight):
    bf16 = ml_dtypes.bfloat16
    x = np.asarray(hidden_states, np.float32).reshape(S, HID)
    pos = np.asarray(position_ids, np.float64).reshape(S)
    q_a_w = np.asarray(q_a_weight, np.float32)
    q_ln = np.asarray(q_a_layernorm_weight, np.float32)
    q_b_w = np.asarray(q_b_weight, np.float32)
    kv_a_w = np.asarray(kv_a_weight, np.float32)
    kv_ln = np.asarray(kv_a_layernorm_weight, np.float32)
    kv_b_w = np.asarray(kv_b_weight, np.float32)
    o_w = np.asarray(o_weight, np.float32)

    wa = np.concatenate([q_a_w, kv_a_w], axis=1).astype(bf16)  # [HID, 2112]
    xT = np.ascontiguousarray(x.T).astype(bf16)                # [HID, S]

    # fold the rms-norm weights into the b-projections
    qb = (q_ln[:, None] * q_b_w).reshape(CQ, H, D_Q)
    kvb = (kv_ln[:, None] * kv_b_w).reshape(CKV, H, D_NOPE + D_V)

    # rope tables
    inv_freq = 1.0 / (10000.0 ** (np.arange(0, D_ROPE, 2, dtype=np.float64) / D_ROPE))
    freqs = pos[:, None] * inv_freq[None, :]                # [S, 32]
    emb = np.concatenate([freqs, freqs], axis=-1)           # [S, 64]
    cos = np.cos(emb).astype(np.float32)
    sin = np.sin(emb).astype(np.float32)
    sin_sg = np.concatenate([-sin[:, :32], sin[:, 32:]], axis=1)  # [S, 64]
    cosT = np.ascontiguousarray(cos.T)                      # [64, S]
    sinT_sg = np.ascontiguousarray(sin_sg.T)                # [64, S]
    cos2t = np.concatenate([cosT, cosT], axis=0)            # [128, S]
    sin2tg = np.concatenate([sinT_sg, sinT_sg], axis=0)     # [128, S]

    # causal masks for the 4 diagonal offsets
    masks = np.zeros((4, 128, 512), np.float32)
    i = np.arange(128)[:, None]
    j = np.arange(512)[None, :]
    for m in range(4):
        masks[m] = ((i + m * 128) <= j).astype(np.float32)
    masks = masks.reshape(512, 512).astype(bf16)

    ones_col = np.ones((128, 1), bf16)
    ones_row = np.ones((1, 128), bf16)

    in_maps = []
    for c in range(N_CORES):
        hs = slice(c * HG, (c + 1) * HG)
        in_maps.append({
            "xT": np.ascontiguousarray(xT[:, c * S_SH:(c + 1) * S_SH]),
            "wa": wa,
            "qbn": np.ascontiguousarray(
                qb[:, hs, :D_NOPE].reshape(CQ, HG * D_NOPE)).astype(bf16),
            "qbp": np.ascontiguousarray(
                qb[:, hs, D_NOPE:].reshape(CQ, HG * D_ROPE)).astype(bf16),
            "kvbk": np.ascontiguousarray(
                kvb[:, hs, :D_NOPE].reshape(CKV, HG * D_NOPE)).astype(bf16),
            "kvbv": np.ascontiguousarray(
                kvb[:, hs, D_NOPE:].reshape(CKV, HG * D_V)).astype(bf16),
            "ow": np.ascontiguousarray(
                o_w[c * HG * D_V:(c + 1) * HG * D_V, :]).astype(bf16),
            "cos_s": np.ascontiguousarray(cos[c * S_SH:(c + 1) * S_SH, :]),
            "sin_sg": np.ascontiguousarray(sin_sg[c * S_SH:(c + 1) * S_SH, :]),
            "cos2t": cos2t,
            "sin2tg": sin2tg,
            "masks": masks,
            "ones_col": ones_col,
            "ones_row": ones_row,
        })
    return in_maps


def kernel(**inputs):
    global LAST_EXEC_NS
    trace = bool(inputs.pop("_trace", False))
    in_maps = _host_inputs(**inputs)
    if "nc" not in _CACHE:
        _CACHE["nc"] = _build_nc()
    nc = _CACHE["nc"]
    res = bass_utils.run_bass_kernel_spmd(
        nc, in_maps, core_ids=list(range(N_CORES)), trace=trace)
    LAST_EXEC_NS = res.exec_time_ns
    total = np.zeros((HID, S), np.float64)
    for c in range(N_CORES):
        total += res.results[c]["outT"].astype(np.float64)
    return np.ascontiguousarray(total.T).astype(np.float32).reshape(1, 1, S, HID)
